# revision 18
# baseline (speedup 1.0000x reference)
"""Trainium2 Bass kernel for nn_Actions_block_14388140442036 (gnn_message_passing).

The reference network is entirely linear (no activations), so the output
    out = segment_sum(actions) @ pol_W + pol_b
collapses to per-effect scalars:
    p[j] = actions[j] @ pol_W  (a dot product against fused weight vectors)
followed by a scalar segment-sum.  Folding pol_W through each branch:

  glob branch:  p_g[i] = (globs @ w1)[U[i]]     + action_globs[i]. w2 + cg
  node branch:  p_n[i] = (nodes @ w3)[V[i]]     + action_nodes[i]. w4 + cn
  edge branch:  p_e[i] = (edges[E[i]] . u1) + (nodes @ wr)[row[E[i]]]
                        + (nodes @ wc)[col[E[i]]] + action_edges[i]. wv + ce

where  w1|w2 = glob_W @ pol_W,  w3|w4 = node_W @ pol_W,
       u1|u2 = e2_W @ pol_W,    wr|wv|wc = e1_W @ u2.

Only ~25% of edge rows are ever referenced (E gathers 100k effects from 400k
edges), so the edge features are gathered on the host (per the sharding
hint: data-parallel over action effects with gathered features) and only the
12.8MB of gathered rows stream through the device.  The nodes table is
needed nearly in full by three different gathers, so it streams once with
three fused weight vectors.

Per core (~15.4MB): large C=16 row-packed DMA tiles (8KB descriptors — the
HWDGE trigger is ~625ns serialized per DMA; small remainder tiles issue
first so their trigger latency hides under the pipeline ramp).  Per 128-row
group the PE transposes the tile (fp32 has no DMA transpose), DVE/ACT
alternate copying 4-group PSUM slabs back to SBUF, and the PE then matmuls
them against the fused weight columns, accumulating dot-product columns
directly in PSUM banks.  The small action-feature matvecs run on the DVE
(mul + 3D-view reduce) in chunks interleaved with the slab copies.  Each
branch's accumulator drains to HBM as soon as its last dots are emitted.
The host does the tiny fused-weight precompute, the scalar gathers and the
segment sum.
"""

import numpy as np

import concourse.bacc as bacc
import concourse.mybir as mybir
import concourse.tile as tile
from concourse.bass_utils import run_bass_kernel_spmd
from concourse.masks import make_identity

# ---- problem constants (hardcoded; kernel.py must be self-contained) ----
HID = 128
FEAT = 16
N_NODES = 100000
N_EDGES = 400000
N_PER = 100000
A_TOTAL = 300000
NUM_ACTIONS = 75000
N_CORES = 8

N_SH = N_NODES // N_CORES   # 12500 node rows per core
A_SH = N_PER // N_CORES     # 12500 action-effect rows per core (all branches)

# Row->SBUF packing: C consecutive rows per partition, so a [128, C*W] tile
# covers 128*C rows with C*W*4B contiguous DMA descriptors.
C = 16              # main DMA tiles [128, 2048]
T_M = 6             # 6*2048 = 12288 main rows per 12500-row stream
M_TAIL = 212        # rows 12288..12500 as [106, 256] (C=2)

C_A = 8             # apack chunks [128, 384] cover 1024 rows (48 floats/row)
T_A = 12            # 12*1024 = 12288 main rows, tail 212 rows -> [106, 96]
A_TAIL = 212

QG_COLS = T_M * C + 2            # 98 groups x 1 col (gathered-edge dots)
QN_COLS = (T_M * C + 2) * 3      # 294: 98 groups x 3 weights
PA_COLS = T_A * C_A * 3 + 6      # 294

# wts input [128, 900]: [0:3]=Wn columns (w3|wr|wc), [4:516]=u1 tiled x4
# replicated across partitions, [516:900]=w48 (=[w2|w4|wv]) tiled x8 replicated
W_N = (0, 3)
W_U1B = (4, 4 + 4 * HID)
W_A48 = (4 + 4 * HID, 4 + 4 * HID + C_A * 48)
WTS_COLS = 4 + 4 * HID + C_A * 48

F32 = mybir.dt.float32
AX = mybir.AxisListType.X

_CACHE = {}


def _build_program(repeat=1):
    nc = bacc.Bacc("TRN2", target_bir_lowering=False, debug=False,
                   num_devices=N_CORES)

    eg_in = nc.dram_tensor("eg_in", [A_SH, HID], F32, kind="ExternalInput").ap()
    nodes_in = nc.dram_tensor("nodes_in", [N_SH, HID], F32, kind="ExternalInput").ap()
    apack_in = nc.dram_tensor("apack_in", [A_SH, 3 * FEAT], F32, kind="ExternalInput").ap()
    wts_in = nc.dram_tensor("wts_in", [128, WTS_COLS], F32, kind="ExternalInput").ap()

    qg_out = nc.dram_tensor("qg_out", [128, QG_COLS], F32, kind="ExternalOutput").ap()
    qn_out = nc.dram_tensor("qn_out", [128, QN_COLS], F32, kind="ExternalOutput").ap()
    pa_out = nc.dram_tensor("pa_out", [128, PA_COLS], F32, kind="ExternalOutput").ap()

    with tile.TileContext(nc) as tc:
        with (
            tc.tile_pool(name="wpool", bufs=1) as wpool,
            tc.tile_pool(name="dpool", bufs=6) as dpool,
            tc.tile_pool(name="adpool", bufs=3) as adpool,
            tc.tile_pool(name="dtpool", bufs=6) as dtpool,
            tc.tile_pool(name="atpool", bufs=2) as atpool,
            tc.tile_pool(name="accpool", bufs=1) as accpool,
            tc.tile_pool(name="pstr", bufs=5, space="PSUM") as pstr,
            tc.tile_pool(name="psacc", bufs=1, space="PSUM") as psacc,
        ):
            wt = wpool.tile([128, WTS_COLS], F32)
            nc.gpsimd.dma_start(wt[:], wts_in[:])
            ident = wpool.tile([128, 128], F32)
            make_identity(nc, ident[:])
            wn_col = wt[:, W_N[0]:W_N[1]]
            u1b = wt[:, W_U1B[0]:W_U1B[1]]
            a48b = wt[:, W_A48[0]:W_A48[1]]

            qn_ps = psacc.tile([128, QN_COLS], F32)
            qg_sb = accpool.tile([128, QG_COLS], F32)
            pa_acc = accpool.tile([128, PA_COLS], F32)

            g_main = eg_in[0:T_M * 128 * C, :].rearrange(
                "(t p c) f -> t p (c f)", p=128, c=C)
            g_tl = eg_in[T_M * 128 * C:A_SH, :].rearrange("(p c) f -> p (c f)", c=2)
            n_main = nodes_in[0:T_M * 128 * C, :].rearrange(
                "(t p c) f -> t p (c f)", p=128, c=C)
            n_tl = nodes_in[T_M * 128 * C:N_SH, :].rearrange("(p c) f -> p (c f)", c=2)
            # last eg tile split into 4 C=4 sub-tiles so the end-of-stream
            # drain (mul -> reduce -> out DMA) is short
            g_last = eg_in[(T_M - 1) * 128 * C:T_M * 128 * C, :].rearrange(
                "(t p c) f -> t p (c f)", p=128, c=4)

            # tile specs:
            #  ("pe",  src, parts, n_groups, rhs, acc, [cols])  nodes: PE path
            #  ("dve", src, parts, n_groups, base_col)          eg: DVE path
            # Small remainder tiles first (trigger latency hides in the ramp);
            # nodes and eg interleave to balance PE vs DVE over time; eg ends
            # the stream with small split tiles for a short tail.
            tiles = [
                ("dve", g_tl, 106, 2, T_M * C),
                ("pe", n_tl, 106, 2, wn_col, qn_ps,
                 [((T_M * C + g) * 3, 3) for g in range(2)]),
            ]
            for t in range(T_M):
                tiles.append(("pe", n_main[t], 128, C, wn_col, qn_ps,
                              [((t * C + g) * 3, 3) for g in range(C)]))
                if t < T_M - 1:
                    tiles.append(("dve", g_main[t], 128, C, t * C))
            tiles += [("dve", g_last[q], 128, 4, (T_M - 1) * C + q * 4)
                      for q in range(4)]
            n_last_pe = max(i for i, t in enumerate(tiles) if t[0] == "pe")
            n_last_dve = max(i for i, t in enumerate(tiles) if t[0] == "dve")

            # ---- action-feature chunks (DVE mul + 3D-view reduce) ----
            a_main = apack_in[0:T_A * 128 * C_A, :].rearrange(
                "(t p c) f -> t p (c f)", p=128, c=C_A)
            a_tl = apack_in[T_A * 128 * C_A:A_SH, :].rearrange(
                "(p c) f -> p (c f)", c=2)

            def emit_action_chunk(t):
                if t < T_A:
                    d = adpool.tile([128, C_A * 48], F32, tag="ad")
                    nc.sync.dma_start(d[:], a_main[t])
                    tmp = atpool.tile([128, C_A * 48], F32, tag="at")
                    nc.vector.tensor_mul(tmp[:], d[:], a48b)
                    nc.vector.reduce_sum(
                        pa_acc[:, t * C_A * 3:(t + 1) * C_A * 3],
                        tmp[:].rearrange("p (s f) -> p s f", f=FEAT), axis=AX)
                else:
                    ap_t = A_TAIL // 2  # 106
                    d = adpool.tile([128, 96], F32, tag="ad")
                    nc.sync.dma_start(d[:ap_t, :], a_tl)
                    tmp = atpool.tile([128, 96], F32, tag="at")
                    nc.vector.tensor_mul(tmp[:ap_t, :], d[:ap_t, :], a48b[:ap_t, :96])
                    nc.vector.reduce_sum(
                        pa_acc[:ap_t, T_A * C_A * 3:T_A * C_A * 3 + 6],
                        tmp[:ap_t, :].rearrange("p (s f) -> p s f", f=FEAT), axis=AX)
                if t == T_A:
                    nc.sync.dma_start(pa_out[:], pa_acc[:])

            # nodes: 2-slab software pipeline (dots two slabs behind the
            # transposes); node slab copies all on ACT (DVE owns eg+actions).
            pending = []
            state = {"unit": 0, "action": 0}

            def emit_dots():
                parts, gs, rhs, acc, cols, dT, last = pending.pop(0)
                for g in range(gs):
                    c0, ncol = cols[g]
                    nc.tensor.matmul(
                        acc[:parts, c0:c0 + ncol],
                        dT[:, g * 128:g * 128 + parts],
                        rhs[:, :])
                if last:
                    sb = accpool.tile([128, QN_COLS], F32, tag="qnsb")
                    nc.scalar.copy(sb[:], acc[:])
                    nc.sync.dma_start(qn_out[:], sb[:])

            def tick():
                state["unit"] += 1
                if state["unit"] % 4 == 0 and state["action"] <= T_A:
                    emit_action_chunk(state["action"])
                    state["action"] += 1

            for _rep in range(repeat):
                state["action"] = 0
                for ti, spec in enumerate(tiles):
                    kind, src, parts, n_groups = spec[0], spec[1], spec[2], spec[3]
                    d = dpool.tile([128, C * HID], F32, tag="d")
                    nc.sync.dma_start(d[:parts, :n_groups * HID], src)
                    if kind == "pe":
                        rhs, acc, cols = spec[4], spec[5], spec[6]
                        for sl in range(0, n_groups, 4):
                            gs = min(4, n_groups - sl)
                            ps = pstr.tile([128, 512], F32, tag="ps")
                            for g in range(gs):
                                nc.tensor.transpose(
                                    ps[:, g * 128:g * 128 + parts],
                                    d[:parts, (sl + g) * 128:(sl + g + 1) * 128],
                                    ident[:parts, :parts])
                            dT = dtpool.tile([128, 512], F32, tag="dT")
                            nc.scalar.copy(dT[:, :gs * 128], ps[:, :gs * 128])
                            last = (ti == n_last_pe) and sl + 4 >= n_groups
                            pending.append((parts, gs, rhs, acc, cols[sl:sl + gs], last and True or False, last))
                            pending[-1] = (parts, gs, rhs, acc, cols[sl:sl + gs], dT, last)
                            if len(pending) > 2:
                                emit_dots()
                            tick()
                    else:
                        base = spec[4]
                        for sl in range(0, n_groups, 4):
                            gs = min(4, n_groups - sl)
                            tmp = atpool.tile([128, 512], F32, tag="egt")
                            nc.vector.tensor_mul(
                                tmp[:parts, :gs * 128],
                                d[:parts, sl * 128:(sl + gs) * 128],
                                u1b[:parts, :gs * 128])
                            nc.vector.reduce_sum(
                                qg_sb[:parts, base + sl:base + sl + gs],
                                tmp[:parts, :gs * 128].rearrange(
                                    "p (c f) -> p c f", f=HID), axis=AX)
                            if ti == n_last_dve and sl + 4 >= n_groups:
                                nc.sync.dma_start(qg_out[:], qg_sb[:])
                            tick()
                while pending:
                    emit_dots()
                while state["action"] <= T_A:
                    emit_action_chunk(state["action"])
                    state["action"] += 1

    nc.compile()
    return nc


def _get_program():
    if "nc" not in _CACHE:
        _CACHE["nc"] = _build_program()
    return _CACHE["nc"]


def _unscramble_q1(arr):
    """[128, 98] -> [12500] (gathered-edge dots) in original row order."""
    tm = T_M - 1
    main = arr[:, :tm * C].reshape(128, tm, C).transpose(1, 0, 2).reshape(-1)
    # last main tile was emitted as 4 C=4 sub-tiles: rows 10240+q*512+p*4+g
    split = arr[:, tm * C:T_M * C].reshape(128, 4, 4).transpose(1, 0, 2).reshape(-1)
    tail = arr[:106, T_M * C:].reshape(-1)                 # rows 12288 + p*2+g
    return np.concatenate([main, split, tail])


def _unscramble_qn(arr):
    """[128, 294] -> [12500, 3] (w3, wr, wc dots) in original row order."""
    main = arr[:, :T_M * C * 3].reshape(128, T_M, C, 3).transpose(1, 0, 2, 3)
    main = main.reshape(-1, 3)                             # rows t*2048+p*16+g
    tail = arr[:106, T_M * C * 3:].reshape(106, 2, 3).reshape(-1, 3)
    return np.concatenate([main, tail], axis=0)


def _unscramble_pa(arr):
    """[128, 294] -> [12500, 3] (ag.w2, an.w4, ae.wv) in original row order."""
    main = arr[:, :T_A * C_A * 3].reshape(128, T_A, C_A, 3).transpose(1, 0, 2, 3)
    main = main.reshape(-1, 3)                             # rows t*1024+p*8+j
    tail = arr[:A_TAIL // 2, T_A * C_A * 3:].reshape(106, 2, 3).reshape(-1, 3)
    return np.concatenate([main, tail], axis=0)


def kernel(**inputs):
    inputs = {k: np.asarray(v) for k, v in inputs.items()}
    globs = inputs["globs"]
    nodes = np.ascontiguousarray(inputs["nodes"])
    edges = np.ascontiguousarray(inputs["edges"])
    action_globs = inputs["action_globs"]
    action_nodes = inputs["action_nodes"]
    action_edges = inputs["action_edges"]
    glob_W = inputs["glob_W"]; glob_b = inputs["glob_b"]
    node_W = inputs["node_W"]; node_b = inputs["node_b"]
    e1_W = inputs["e1_W"]; e1_b = inputs["e1_b"]
    e2_W = inputs["e2_W"]; e2_b = inputs["e2_b"]
    pol_W = inputs["pol_W"]; pol_b = inputs["pol_b"]
    row = inputs["row"]; col = inputs["col"]
    U = inputs["U"]; UA = inputs["UA"]; V = inputs["V"]; VA = inputs["VA"]
    E = inputs["E"]; EA = inputs["EA"]
    actions_batch = inputs["actions_batch"]

    # ---- fused weight vectors (float64 for accuracy; cast to f32 on device) ----
    polW = pol_W.astype(np.float64)[:, 0]                 # [128]
    g_f = glob_W.astype(np.float64) @ polW                # [144]
    n_f = node_W.astype(np.float64) @ polW                # [144]
    e2_f = e2_W.astype(np.float64) @ polW                 # [256]
    u1, u2 = e2_f[:HID], e2_f[HID:]
    e1_f = e1_W.astype(np.float64) @ u2                   # [272]
    w1, w2 = g_f[:HID], g_f[HID:]
    w3, w4 = n_f[:HID], n_f[HID:]
    wr, wv, wc = e1_f[:HID], e1_f[HID:HID + FEAT], e1_f[HID + FEAT:]
    cg = float(glob_b.astype(np.float64) @ polW)
    cn = float(node_b.astype(np.float64) @ polW)
    ce = float(e2_b.astype(np.float64) @ polW + e1_b.astype(np.float64) @ u2)

    wts = np.zeros((128, WTS_COLS), np.float32)
    wts[:, W_N[0]] = w3.astype(np.float32)
    wts[:, W_N[0] + 1] = wr.astype(np.float32)
    wts[:, W_N[0] + 2] = wc.astype(np.float32)
    wts[:, W_U1B[0]:W_U1B[1]] = np.tile(u1.astype(np.float32), (128, 4))
    w48 = np.concatenate([w2, w4, wv]).astype(np.float32)
    wts[:, W_A48[0]:W_A48[1]] = np.tile(w48, (128, C_A))

    # gathered edge features for the edge branch (only ~25% of edge rows are
    # referenced; shipping the gathered rows quarters the edge stream)
    eg = edges[E]                                          # [N_PER, 128]

    # packed action features [N_PER, 48] = [ag | an | ae]
    apack = np.empty((N_PER, 3 * FEAT), np.float32)
    apack[:, :FEAT] = action_globs
    apack[:, FEAT:2 * FEAT] = action_nodes
    apack[:, 2 * FEAT:] = action_edges

    nc = _get_program()
    in_maps = []
    for c in range(N_CORES):
        in_maps.append({
            "eg_in": eg[c * A_SH:(c + 1) * A_SH],
            "nodes_in": nodes[c * N_SH:(c + 1) * N_SH],
            "apack_in": apack[c * A_SH:(c + 1) * A_SH],
            "wts_in": wts,
        })
    res = run_bass_kernel_spmd(nc, in_maps, core_ids=list(range(N_CORES)))

    qe_g = np.empty(N_PER, np.float64)                    # edges[E].u1, effect order
    qn3 = np.empty((N_NODES, 3), np.float64)
    pa = np.empty((N_PER, 3), np.float64)
    for c in range(N_CORES):
        r = res.results[c]
        qe_g[c * A_SH:(c + 1) * A_SH] = _unscramble_q1(r["qg_out"])
        qn3[c * N_SH:(c + 1) * N_SH] = _unscramble_qn(r["qn_out"])
        pa[c * A_SH:(c + 1) * A_SH] = _unscramble_pa(r["pa_out"])
    qn, qr, qc = qn3[:, 0], qn3[:, 1], qn3[:, 2]

    # ---- host: gathers, scatter into action slots, segment sum ----
    qg = globs.astype(np.float64) @ w1                    # [512]
    p_g = qg[U] + pa[:, 0] + cg
    p_n = qn[V] + pa[:, 1] + cn
    p_e = qe_g + qr[row[E]] + qc[col[E]] + pa[:, 2] + ce

    actions_p = np.zeros(A_TOTAL, np.float64)
    actions_p[UA] = p_g
    actions_p[VA] = p_n
    actions_p[EA] = p_e

    # torch-style _norm: consecutive group ids starting at actions_batch[0]
    ab = actions_batch.astype(np.int64)
    changed = ab[1:] != ab[:-1]
    seg = int(ab[0]) + np.concatenate([[0], np.cumsum(changed)])
    if seg[0] >= 0 and seg[-1] < NUM_ACTIONS:
        agg = np.bincount(seg, weights=actions_p, minlength=NUM_ACTIONS)[:NUM_ACTIONS]
    else:  # jax segment_sum drops out-of-range ids
        agg = np.zeros(NUM_ACTIONS, np.float64)
        valid = (seg >= 0) & (seg < NUM_ACTIONS)
        np.add.at(agg, seg[valid], actions_p[valid])

    out = agg + float(pol_b.astype(np.float64)[0])
    return out.astype(np.float32)[:, None]
